# revision 1
# baseline (speedup 1.0000x reference)
"""Trainium2 Bass kernel for nn_Attention_72541997629647 (sparse varlen attention).

Computation (see problem reference):
  qkv = x @ w_qkv.T + b_qkv ; NeoX RoPE on q,k ; block-diagonal softmax
  attention from cu_seqlens segments ; out = (attn @ v) @ w_proj.T + b_proj

Sharding: tensor-parallel over heads. 16 heads / 8 cores = 2 heads per core.
Each core computes q/k/v for its 2 heads, runs block-diagonal attention, and
produces a partial projection output (full [DIM, S], transposed); the host
sums the 8 partials and adds b_proj, so the result is exact.

Device dataflow per core (all matmuls in float32r: full fp32 storage,
reduced-precision multiply at 4x the fp32 matmul rate):
  A) QKV: out_nat[s, 480] = xT-chunks.T @ w_chunks (+ bias via ones-row
     matmul); RoPE applied on the free dim (half-swap via negative-step AP,
     sign folded into the host-built sin table); q,k PE-transposed to
     [hd, S]; v kept natural with an appended ones column (denominator trick).
  B) per (head, segment, q-chunk): scoresT[k,q] = kT-block.T @ qT ; exp on
     ACT ; attn_extT[81, q] += v_ext.T @ exp accumulated over k-blocks; row 80
     is the softmax denominator. normalize = reciprocal + ones-matmul
     partition-broadcast + multiply.
  C) proj: outT[dim, s] += wpT-head.T @ attn_outT-head ; PSUM->SBUF copy on
     DVE; output written as fully-contiguous 1.25MB blocks (one dense
     descriptor chain per DMA, ~70us faster than 2KB-strided rows) and
     unscrambled on the host. b_proj is added host-side.
"""

import os
import sys

for _p in ("/opt/trn_rl_repo", "/root/.axon_site/_ro/trn_rl_repo"):
    if os.path.isdir(_p) and _p not in sys.path:
        sys.path.insert(0, _p)

import numpy as np

import concourse.bacc as bacc
import concourse.bass as bass
import concourse.mybir as mybir
import concourse.tile as tile
from concourse.bass_utils import run_bass_kernel_spmd
from contextlib import ExitStack

S = 3072
DIM = 1280
H = 16
HD = 80
NCORES = 8
HPC = H // NCORES          # heads per core = 2
QKDIM = 2 * HPC * HD       # 320 (q+k outdims per core)
ODIM = 3 * HPC * HD        # 480 (qkv outdims per core)
CDIM = HPC * HD            # 160 (attn channels per core)

F32 = mybir.dt.float32
F32R = mybir.dt.float32r
MM_DT = F32R               # matmul input dtype (F32R: 4x faster, ~1e-4 rel err)

_CACHE: dict = {}


def _segments_from_cu(cu_seqlens: np.ndarray) -> tuple:
    """Contiguous runs of equal segment id, exactly as the reference's
    searchsorted-based mask defines them."""
    cu = np.asarray(cu_seqlens).astype(np.int64)
    seg = np.searchsorted(cu, np.arange(S), side="right") - 1
    change = np.nonzero(np.diff(seg))[0]
    starts = np.concatenate([[0], change + 1])
    ends = np.concatenate([change + 1, [S]])
    return tuple((int(a), int(b)) for a, b in zip(starts, ends))


def _build(segments, loop_n: int = 1) -> "bacc.Bacc":
    nc = bacc.Bacc("TRN2", target_bir_lowering=False, debug=False,
                   num_devices=NCORES)

    xblk_d = nc.dram_tensor("xblk", [S // 512, 5, 128, 2, 512], MM_DT,
                        kind="ExternalInput")
    wqkvT_d = nc.dram_tensor("wqkvT", [DIM, ODIM], MM_DT, kind="ExternalInput")
    bqkv_d = nc.dram_tensor("bqkv", [1, ODIM], MM_DT, kind="ExternalInput")
    cosb_d = nc.dram_tensor("cosb", [S // 512, 128, 4, HD], F32,
                        kind="ExternalInput")
    sinb_d = nc.dram_tensor("sinb", [S // 512, 128, 4, HD], F32,
                        kind="ExternalInput")
    wpT_d = nc.dram_tensor("wpT", [CDIM, DIM], MM_DT, kind="ExternalInput")
    ident_d = nc.dram_tensor("ident", [128, 128], MM_DT, kind="ExternalInput")
    ones_d = nc.dram_tensor("onesrow", [1, 128], MM_DT, kind="ExternalInput")
    vpad_d = nc.dram_tensor("vpad", [17], MM_DT, kind="ExternalInput")
    # boundary-block 0/1 masks (segments not aligned to the 128 grid);
    # order must match the (head-agnostic) traversal below.
    bpairs = []
    for (s0, s1) in segments:
        for j in range(s0 // 128, -(-s1 // 128)):
            r0, r1 = max(0, s0 - 128 * j), min(128, s1 - 128 * j)
            if r0 > 0 or r1 < 128:
                bpairs.append((j, r0, r1))
    nbm = len(bpairs)
    bmask_d = (nc.dram_tensor("bmask", [nbm, 128], MM_DT, kind="ExternalInput")
               if nbm else None)
    outb_d = nc.dram_tensor("outb", [S // 512, 2, 128, 5, 512], F32,
                        kind="ExternalOutput")

    NT = S // 128   # 24 s-tiles
    NSS = S // 512  # 6 s-superchunks

    with tile.TileContext(nc) as tc, ExitStack() as ctx:
        if loop_n > 1:  # benchmarking only: repeat the whole body on-device
            ctx.enter_context(tc.For_i(0, loop_n, 1))
        per = ctx.enter_context(tc.tile_pool(name="persist", bufs=1))

        # small constants first so nothing cheap blocks the pipeline
        bqkv_sb = per.tile([1, ODIM], MM_DT, tag="bqkv")
        nc.sync.dma_start(out=bqkv_sb, in_=bqkv_d[:, :])
        ident_sb = per.tile([128, 128], MM_DT, tag="ident")
        nc.sync.dma_start(out=ident_sb, in_=ident_d[:, :])
        ones_sb = per.tile([1, 128], MM_DT, tag="ones")
        nc.sync.dma_start(out=ones_sb, in_=ones_d[:, :])
        # per-d-chunk qkv weights and per-superchunk rope tables: split so the
        # first matmul/rope can start after a fraction of the weight traffic
        wqkv_sb = [per.tile([128, ODIM], MM_DT, tag=f"wqkv{d}", name=f"wqkv{d}")
                   for d in range(10)]
        cos_sb = [per.tile([128, 4, HD], F32, tag=f"cos{ss}", name=f"cos{ss}")
                  for ss in range(NSS)]
        sin_sb = [per.tile([128, 4, HD], F32, tag=f"sin{ss}", name=f"sin{ss}")
                  for ss in range(NSS)]
        wp_sb = [per.tile([HD, DIM], MM_DT, tag=f"wp{h}", name=f"wp{h}") for h in range(HPC)]
        for h in range(HPC):
            nc.sync.dma_start(out=wp_sb[h], in_=wpT_d[h * HD:(h + 1) * HD, :])

        # v extended to 97 cols: 80 v-dims, 16 zero pad, ones col at 96 so the
        # denominator lands on a 32-aligned PSUM partition. Split per 512-s
        # superchunk so attention can start before all of phase A finishes.
        VEXT = 97
        v_sb = [[per.tile([128, 4, VEXT], MM_DT, tag=f"v{h}_{ss}",
                          name=f"v{h}_{ss}") for ss in range(NSS)]
                for h in range(HPC)]
        qkT = [[per.tile([HD, 512], MM_DT, tag=f"qkT{j}_{ss}",
                         name=f"qkT{j}_{ss}") for ss in range(NSS)]
               for j in range(2 * HPC)]
        att_o = [[per.tile([HD, 512], MM_DT, tag=f"atto{h}_{ss}",
                           name=f"atto{h}_{ss}") for ss in range(NSS)]
                 for h in range(HPC)]

        # one shared PSUM pool (8 bank-sized slots shared by every phase so
        # the scheduler can overlap A/B/C), plus top-level SBUF pools
        psp = ctx.enter_context(tc.tile_pool(name="ps", bufs=8, space="PSUM"))
        xtp = ctx.enter_context(tc.tile_pool(name="xt", bufs=6))
        ropep = ctx.enter_context(tc.tile_pool(name="ropet", bufs=2))
        qkrop = ctx.enter_context(tc.tile_pool(name="qkro", bufs=3))
        expp = ctx.enter_context(tc.tile_pool(name="expp", bufs=5))
        smp = ctx.enter_context(tc.tile_pool(name="smalls", bufs=2))
        outp = ctx.enter_context(tc.tile_pool(name="outp", bufs=2))

        if nbm:
            bmask_sb = per.tile([128, nbm], MM_DT, tag="bmask")
            nc.sync.dma_start(out=bmask_sb,
                              in_=bmask_d.ap().rearrange("n p -> p n"))
            bidx = {(j, r0, r1): i for i, (j, r0, r1) in enumerate(bpairs)}

        # ---------------- phase bodies (emitted interleaved below) --------
        def emit_A(ss):
            """QKV + RoPE + transposes for s-superchunk ss."""
            xts = []
            for dp in range(5):
                if ss == 0:
                    for d in (2 * dp, 2 * dp + 1):
                        nc.sync.dma_start(
                            out=wqkv_sb[d],
                            in_=wqkvT_d[128 * d:128 * (d + 1), :])
                xt = xtp.tile([128, 2, 512], MM_DT, tag="xt", name="xt")
                nc.sync.dma_start(out=xt, in_=xblk_d[ss, dp])
                xts.append(xt)
            nc.sync.dma_start(out=cos_sb[ss], in_=cosb_d[ss])
            nc.sync.dma_start(out=sin_sb[ss], in_=sinb_d[ss])
            for h in range(HPC):
                nc.sync.dma_start(
                    out=v_sb[h][ss][:, :, HD:VEXT],
                    in_=bass.AP(tensor=vpad_d, offset=0,
                                ap=[[0, 128], [0, 4], [1, VEXT - HD]]))
            tp_ps = [psp.tile([HD, 512], MM_DT, tag="ps", name="tpps")
                     for _ in range(2 * HPC)]
            nh = 2 * HPC  # 4 roped qk tensor-heads
            for sub in range(4):
                qp = psp.tile([128, ODIM], F32, tag="ps", name="qkvps")
                for d in range(10):
                    nc.tensor.matmul(
                        qp[:, :],
                        lhsT=xts[d // 2][:, d % 2, 128 * sub:128 * (sub + 1)],
                        rhs=wqkv_sb[d], start=(d == 0), stop=False)
                nc.tensor.matmul(qp[:, :], lhsT=ones_sb[:, :],
                                 rhs=bqkv_sb[:, :], start=False, stop=True)

                # RoPE over q,k: out = t*cos + halfswap(t)*sinsgn
                m1 = ropep.tile([128, QKDIM], F32, tag="m1")
                m2 = ropep.tile([128, QKDIM], F32, tag="m2")
                qk_h = qp[:, 0:QKDIM].rearrange("p (h d) -> p h d", h=nh)
                cos_b = cos_sb[ss][:, sub:sub + 1, :].to_broadcast(
                    [128, nh, HD])
                nc.vector.tensor_mul(
                    m1.rearrange("p (h d) -> p h d", h=nh), qk_h, cos_b)
                swap = qp[:, 0:QKDIM].rearrange(
                    "p (h x d) -> p h x d", h=nh, x=2)[:, :, ::-1, :]
                sin_b = sin_sb[ss][:, sub:sub + 1, :].rearrange(
                    "p t (x d) -> p (t x) d", x=2)[:, None, :, :] \
                    .to_broadcast([128, nh, 2, HD // 2])
                nc.vector.tensor_mul(
                    m2.rearrange("p (h x d) -> p h x d", h=nh, x=2),
                    swap, sin_b)
                ro = qkrop.tile([128, QKDIM], MM_DT, tag="qkro")
                with nc.allow_low_precision("f32r matmul inputs"):
                    nc.vector.tensor_add(ro, m1, m2)

                # v natural copy (its bias already in psum)
                for h in range(HPC):
                    nc.scalar.copy(
                        v_sb[h][ss][:, sub, 0:HD],
                        qp[:, QKDIM + HD * h:QKDIM + HD * (h + 1)])

                # transpose roped q,k to [hd, s]
                for j in range(2 * HPC):
                    nc.tensor.transpose(
                        tp_ps[j][:, 128 * sub:128 * (sub + 1)],
                        ro[:, HD * j:HD * (j + 1)], ident_sb)
            for j in range(2 * HPC):
                nc.scalar.copy(qkT[j][ss], tp_ps[j])

        def emit_B(seg):
            """block-diagonal attention for one segment (both heads)."""
            s0, s1 = seg
            jb0, jb1 = s0 // 128, -(-s1 // 128)
            # q-chunks aligned to the 512 grid so each lives in one tile
            g = (s0 // 512) * 512
            qchunks = []
            while g < s1:
                q0, q1 = max(s0, g), min(s1, g + 512)
                if q1 > q0:
                    qchunks.append((q0, q1))
                g += 512
            for q0, q1 in qchunks:
                qn = q1 - q0
                ss_q, c0 = q0 // 512, q0 % 512
                # fp32r matmuls need an even/4-aligned moving dim: widen the
                # compute window to 4-aligned columns (scratch cols unread)
                qa0 = q0 - (q0 % 4)
                qa1 = min(512 * (ss_q + 1), q1 + ((-q1) % 4))
                qna, off, ca0 = qa1 - qa0, q0 - qa0, qa0 % 512
                for h in range(HPC):
                    ap_ = psp.tile([VEXT, 512], F32, tag="ps", name="attps")
                    blocks = list(range(jb0, jb1))
                    for g0 in range(0, len(blocks), 4):
                        grp = blocks[g0:g0 + 4]
                        exs = []
                        for j in grp:
                            kTt = qkT[HPC + h][j // 4]
                            sc = psp.tile([128, 512], F32, tag="ps",
                                          name="scps")
                            nc.tensor.matmul(
                                sc[:, :qna],
                                lhsT=kTt[:, 128 * (j % 4):128 * (j % 4 + 1)],
                                rhs=qkT[h][ss_q][:, ca0:ca0 + qna],
                                start=True, stop=True)
                            ex = expp.tile([128, 512], MM_DT, tag="expp")
                            nc.scalar.activation(
                                ex[:, :qna], sc[:, :qna],
                                mybir.ActivationFunctionType.Exp)
                            r0, r1 = max(0, s0 - 128 * j), min(128, s1 - 128 * j)
                            if r0 > 0 or r1 < 128:
                                # zero out-of-segment rows of this block
                                mi = bidx[(j, r0, r1)]
                                with nc.allow_low_precision("f32r inputs"):
                                    nc.vector.tensor_mul(
                                        ex[:, :qna], ex[:, :qna],
                                        bmask_sb[:, mi:mi + 1]
                                        .to_broadcast([128, qna]))
                            exs.append(ex)
                        for j, ex in zip(grp, exs):
                            nc.tensor.matmul(
                                ap_[:, :qna],
                                lhsT=v_sb[h][j // 4][:, j % 4, :],
                                rhs=ex[:, :qna],
                                start=(j == blocks[0]),
                                stop=(j == blocks[-1]))
                    den = smp.tile([1, 512], MM_DT, tag="den", name="den")
                    with nc.allow_low_precision("f32r matmul inputs"):
                        nc.scalar.copy(den[:, :qna], ap_[96:97, :qna])
                    bc = psp.tile([HD, 512], F32, tag="ps", name="bcps")
                    nc.tensor.matmul(bc[:, :qna], lhsT=ones_sb[:, 0:HD],
                                     rhs=den[:, :qna], start=True, stop=True)
                    rec = smp.tile([HD, 512], F32, tag="rec", name="rec")
                    nc.vector.reciprocal(rec[:, :qna], bc[:, :qna])
                    with nc.allow_low_precision("f32r matmul inputs"):
                        nc.vector.tensor_mul(att_o[h][ss_q][:, c0:c0 + qn],
                                             ap_[0:HD, off:off + qn],
                                             rec[:, off:off + qn])

        def emit_C(sc_):
            """projection for output s-superchunk sc_.
            b_proj is added host-side after the cross-core partial sum."""
            for mh in range(2):
                ob = outp.tile([128, 5, 512], F32, tag="outp")
                for mm_ in range(5):
                    m = 5 * mh + mm_
                    pp = psp.tile([128, 512], F32, tag="ps", name="prps")
                    for h in range(HPC):
                        nc.tensor.matmul(
                            pp[:, :],
                            lhsT=wp_sb[h][:, 128 * m:128 * (m + 1)],
                            rhs=att_o[h][sc_],
                            start=(h == 0), stop=(h == HPC - 1))
                    nc.vector.tensor_copy(ob[:, mm_, :], pp)
                nc.sync.dma_start(out=outb_d[sc_, mh], in_=ob)

        # ---- interleaved driver: emit B as soon as its span is produced,
        # ---- C as soon as all segments covering its chunk are attended.
        segs_left = sorted(segments, key=lambda s: s[1])
        segs_done: list = []
        c_next = 0
        for ss in range(NSS):
            emit_A(ss)
            done_to = 512 * (ss + 1)
            while segs_left and segs_left[0][1] <= done_to:
                seg = segs_left.pop(0)
                emit_B(seg)
                segs_done.append(seg)
            covered = min((s0 for (s0, s1) in segs_left), default=S)
            while c_next < NSS and 512 * (c_next + 1) <= covered:
                emit_C(c_next)
                c_next += 1
        assert not segs_left
        while c_next < NSS:
            emit_C(c_next)
            c_next += 1

    nc.compile()
    return nc


def _prep_inputs(x, cu_seqlens, rotary_pos_emb, w_qkv, b_qkv, w_proj, b_proj):
    """Host-side shard prep. Returns per-core input dicts."""
    scale = np.float32(1.0 / np.sqrt(np.float32(HD)))
    xT = np.ascontiguousarray(np.asarray(x, np.float32).T)
    w_qkv = np.asarray(w_qkv, np.float32)
    b_qkv = np.asarray(b_qkv, np.float32)
    w_proj = np.asarray(w_proj, np.float32)
    b_proj = np.asarray(b_proj, np.float32)
    rot = np.asarray(rotary_pos_emb, np.float32)

    cosw = np.concatenate([np.cos(rot), np.cos(rot)], axis=1).astype(np.float32)
    sinw = np.concatenate([-np.sin(rot), np.sin(rot)], axis=1).astype(np.float32)
    # blocked layouts so every device DMA reads one dense contiguous region:
    # xblk[ss,dp,p,c,n] = xT[256dp+128c+p, 512ss+n]; cosb[ss,p,t,d] likewise
    xblk = np.ascontiguousarray(
        xT.reshape(5, 2, 128, 6, 512).transpose(3, 0, 2, 1, 4))
    cosb = np.ascontiguousarray(
        cosw.reshape(6, 4, 128, HD).transpose(0, 2, 1, 3))
    sinb = np.ascontiguousarray(
        sinw.reshape(6, 4, 128, HD).transpose(0, 2, 1, 3))
    ident = np.eye(128, dtype=np.float32)
    onesrow = np.ones((1, 128), dtype=np.float32)
    vpad = np.zeros(17, dtype=np.float32)
    vpad[16] = 1.0
    segments = _segments_from_cu(cu_seqlens)
    bmask_rows = []
    for (s0, s1) in segments:
        for j in range(s0 // 128, -(-s1 // 128)):
            r0, r1 = max(0, s0 - 128 * j), min(128, s1 - 128 * j)
            if r0 > 0 or r1 < 128:
                row = np.zeros(128, dtype=np.float32)
                row[r0:r1] = 1.0
                bmask_rows.append(row)
    bmask = np.stack(bmask_rows) if bmask_rows else None

    in_maps = []
    for c in range(NCORES):
        heads = [HPC * c + i for i in range(HPC)]
        idx = []
        for base in (0, DIM, 2 * DIM):           # q, k, v row blocks
            for h in heads:
                idx.extend(range(base + h * HD, base + (h + 1) * HD))
        w_c = w_qkv[idx, :].copy()
        b_c = b_qkv[idx].copy()
        w_c[:QKDIM // 2] *= scale                # scale q by 1/sqrt(HD)
        b_c[:QKDIM // 2] *= scale
        cdims = []
        for h in heads:
            cdims.extend(range(h * HD, (h + 1) * HD))
        wpT = np.ascontiguousarray(w_proj[:, cdims].T)  # [CDIM, DIM]
        in_maps.append({
            "xblk": xblk,
            "wqkvT": np.ascontiguousarray(w_c.T),
            "bqkv": np.ascontiguousarray(b_c[None, :]),
            "cosb": cosb,
            "sinb": sinb,
            "wpT": wpT,
            "ident": ident,
            "onesrow": onesrow,
            "vpad": vpad,
        })
        if bmask is not None:
            in_maps[-1]["bmask"] = bmask
    return in_maps


def run(inputs: dict, trace: bool = False):
    """Build (cached), run on 8 cores, return (out [S, DIM] fp32, results)."""
    segments = _segments_from_cu(inputs["cu_seqlens"])
    key = (segments, str(MM_DT))
    if key not in _CACHE:
        _CACHE[key] = _build(segments)
    nc = _CACHE[key]
    in_maps = _prep_inputs(
        inputs["x"], inputs["cu_seqlens"], inputs["rotary_pos_emb"],
        inputs["w_qkv"], inputs["b_qkv"], inputs["w_proj"], inputs["b_proj"])
    res = run_bass_kernel_spmd(nc, in_maps, core_ids=list(range(NCORES)),
                               trace=trace)
    acc = np.zeros((DIM, S), np.float64)
    for r in res.results:
        # blocked [sc, mh, p, c, n] -> [dim = 640*mh+128*c+p, s = 512*sc+n]
        acc += r["outb"].transpose(1, 3, 2, 0, 4).reshape(DIM, S)
    acc += np.asarray(inputs["b_proj"], np.float64)[:, None]
    out = np.ascontiguousarray(acc.T.astype(np.float32))
    return out, res


def kernel(**inputs) -> np.ndarray:
    out, _ = run(inputs, trace=False)
    return out



# revision 30
# speedup vs baseline: 2722.9306x; 2722.9306x over previous
"""Trainium2 Bass kernel for nn_Attention_72541997629647 (sparse varlen attention).

Computation (see problem reference):
  qkv = x @ w_qkv.T + b_qkv ; NeoX RoPE on q,k ; block-diagonal softmax
  attention from cu_seqlens segments ; out = (attn @ v) @ w_proj.T + b_proj

Sharding: tensor-parallel over heads. 16 heads / 8 cores = 2 heads per core.
Each core computes q/k/v for its 2 heads, runs block-diagonal attention, and
produces a partial projection output (full [DIM, S], transposed); the host
sums the 8 partials and adds b_proj, so the result is exact.

All matmul inputs and all DMA payloads are bf16 (PSUM accumulation stays
fp32); this halves HBM traffic and DVE elementwise cost vs fp32 while staying
well inside the 2e-2 tolerance.

Device dataflow per core:
  A) QKV: out_nat[s, 480] = xT-chunks.T @ w_chunks (+ bias via ones-row
     matmul); one ACT copy evacuates each PSUM group to bf16 SBUF; RoPE runs
     on bf16 SBUF (2x DVE mode, half-swap via negative-step AP, sign folded
     into the host-built sin table); q,k PE-transposed (bf16, two heads per
     PSUM bank) into persistent [hd, S] tiles, evacuated by DVE; v copied
     natural by the Pool engine, with a memset-built ones column at col 96
     (softmax-denominator trick).
  B) per (head, segment, q-chunk<=512): scoresT[k,q] = kT-block.T @ qT; exp
     on ACT -> bf16 SBUF (narrow chunks share a PSUM bank pairwise with one
     merged exp); attT[97, q] += v_ext.T @ exp accumulated over k-blocks;
     row 96 is the denominator. normalize = den copy + ones-matmul
     partition-broadcast + reciprocal + multiply, software-pipelined one
     group behind the matmuls so the PE never waits on the DVE reciprocal.
     PSUM slots are tag-partitioned (qkv accum / transpose banks / the rest)
     so phase B never FIFO-blocks on phase A's banks.
  C) proj: outT[dim, s] += wpT-head.T @ attT-head ; PSUM->SBUF bf16 copies
     split across ACT/DVE; output DMAd as dense bf16 blocks from the Pool
     engine (SWDGE; small SP-issued pieces for the final superchunk so the
     tail transfer is minimal) and unscrambled on the host. b_proj is added
     host-side after the cross-core partial sum.
"""

import os
import sys

for _p in ("/opt/trn_rl_repo", "/root/.axon_site/_ro/trn_rl_repo"):
    if os.path.isdir(_p) and _p not in sys.path:
        sys.path.insert(0, _p)

import numpy as np

import concourse.bacc as bacc
import concourse.bass as bass
import concourse.mybir as mybir
import concourse.tile as tile
from concourse.bass_utils import run_bass_kernel_spmd
from contextlib import ExitStack

S = 3072
DIM = 1280
H = 16
HD = 80
NCORES = 8
HPC = H // NCORES          # heads per core = 2
QKDIM = 2 * HPC * HD       # 320 (q+k outdims per core)
ODIM = 3 * HPC * HD        # 480 (qkv outdims per core)
CDIM = HPC * HD            # 160 (attn channels per core)
VEXT = 97                  # v extended: 80 v-dims, 16 pad, ones col at 96

F32 = mybir.dt.float32
BF16 = mybir.dt.bfloat16
MM_DT = BF16

_CACHE: dict = {}


def _segments_from_cu(cu_seqlens: np.ndarray) -> tuple:
    """Contiguous runs of equal segment id, exactly as the reference's
    searchsorted-based mask defines them."""
    cu = np.asarray(cu_seqlens).astype(np.int64)
    seg = np.searchsorted(cu, np.arange(S), side="right") - 1
    change = np.nonzero(np.diff(seg))[0]
    starts = np.concatenate([[0], change + 1])
    ends = np.concatenate([change + 1, [S]])
    return tuple((int(a), int(b)) for a, b in zip(starts, ends))


def _bpairs(segments):
    """Boundary (block, r0, r1) triples needing a 0/1 row mask, in traversal
    order shared by host prep and device emission."""
    out = []
    for (s0, s1) in segments:
        for j in range(s0 // 128, -(-s1 // 128)):
            r0, r1 = max(0, s0 - 128 * j), min(128, s1 - 128 * j)
            if r0 > 0 or r1 < 128:
                out.append((j, r0, r1))
    return out


def _build(segments, loop_n: int = 1) -> "bacc.Bacc":
    nc = bacc.Bacc("TRN2", target_bir_lowering=False, debug=False,
                   num_devices=NCORES)

    NSS = S // 512  # 6 s-superchunks

    xblk_d = nc.dram_tensor("xblk", [NSS, 4, 128, 5, 2, 128], MM_DT,
                            kind="ExternalInput")
    wblk_d = nc.dram_tensor("wblk", [128, 10, ODIM], MM_DT,
                            kind="ExternalInput")
    bqkv_d = nc.dram_tensor("bqkv", [1, ODIM], MM_DT, kind="ExternalInput")
    cosb_d = nc.dram_tensor("cosb", [128, NSS, 4, HD], MM_DT,
                            kind="ExternalInput")
    sinb_d = nc.dram_tensor("sinb", [128, NSS, 4, HD], MM_DT,
                            kind="ExternalInput")
    wpT_d = nc.dram_tensor("wpT", [CDIM, DIM], MM_DT, kind="ExternalInput")
    ident_d = nc.dram_tensor("ident", [128, 128], MM_DT, kind="ExternalInput")
    ones_d = nc.dram_tensor("onesrow", [1, 128], MM_DT, kind="ExternalInput")
    vpad_d = nc.dram_tensor("vpad", [17], MM_DT, kind="ExternalInput")
    bpairs = _bpairs(segments)
    nbm = len(bpairs)
    bmask_d = (nc.dram_tensor("bmask", [nbm, 128], MM_DT, kind="ExternalInput")
               if nbm else None)
    outb_d = nc.dram_tensor("outb", [NSS, 2, 128, 5, 512], MM_DT,
                            kind="ExternalOutput")

    with tile.TileContext(nc) as tc, ExitStack() as ctx:
        if loop_n > 1:  # benchmarking only: repeat the whole body on-device
            ctx.enter_context(tc.For_i(0, loop_n, 1))
        per = ctx.enter_context(tc.tile_pool(name="persist", bufs=1))

        # qkv weights first (every matmul needs them), then x superchunk 0;
        # everything else is needed later and queues behind them on the
        # shared DMA engines.
        wblk_sb = per.tile([128, 10, ODIM], MM_DT, tag="wblk")
        nc.sync.dma_start(out=wblk_sb[:, 0:5, :], in_=wblk_d[:, 0:5, :])
        xtp = ctx.enter_context(tc.tile_pool(name="xt", bufs=6))
        xts = []
        for ss in range(NSS):
            xts.append(xtp.tile([128, 4, 5, 2, 128], MM_DT, tag="xt",
                                name="xt"))
        nc.sync.dma_start(out=xts[0][:, 0], in_=xblk_d[0, 0])
        nc.sync.dma_start(out=wblk_sb[:, 5:10, :], in_=wblk_d[:, 5:10, :])
        bqkv_sb = per.tile([1, ODIM], MM_DT, tag="bqkv")
        nc.sync.dma_start(out=bqkv_sb, in_=bqkv_d[:, :])
        ones_sb = per.tile([1, 128], MM_DT, tag="ones")
        nc.sync.dma_start(out=ones_sb, in_=ones_d[:, :])
        cos_sb = per.tile([128, NSS, 4, HD], MM_DT, tag="cos")
        nc.sync.dma_start(out=cos_sb, in_=cosb_d.ap())
        sin_sb = per.tile([128, NSS, 4, HD], MM_DT, tag="sin")
        nc.sync.dma_start(out=sin_sb, in_=sinb_d.ap())
        for sub in range(1, 4):
            nc.sync.dma_start(out=xts[0][:, sub], in_=xblk_d[0, sub])
        ident_sb = per.tile([128, 128], MM_DT, tag="ident")
        nc.sync.dma_start(out=ident_sb, in_=ident_d[:, :])
        wp_sb = [per.tile([HD, DIM], MM_DT, tag=f"wp{h}", name=f"wp{h}")
                 for h in range(HPC)]

        # persistent transposed q,k [hd, S] and attention output [hd, S];
        # v natural per 512-superchunk with the ones column for denominators
        qkT = [per.tile([HD, S], MM_DT, tag=f"qkT{j}", name=f"qkT{j}")
               for j in range(2 * HPC)]
        att_o = [per.tile([HD, S], MM_DT, tag=f"atto{h}", name=f"atto{h}")
                 for h in range(HPC)]
        v_sb = [[per.tile([128, 4, VEXT], MM_DT, tag=f"v{h}_{ss}",
                          name=f"v{h}_{ss}") for ss in range(NSS)]
                for h in range(HPC)]

        # one shared PSUM pool (8 bank-sized slots shared by every phase so
        # the scheduler can overlap A/B/C), plus top-level SBUF pools
        psp = ctx.enter_context(tc.tile_pool(name="ps", bufs=5, space="PSUM"))
        qkvp = ctx.enter_context(tc.tile_pool(name="qkvsb", bufs=3))
        ropep = ctx.enter_context(tc.tile_pool(name="ropet", bufs=2))
        qkrop = ctx.enter_context(tc.tile_pool(name="qkro", bufs=3))
        expp = ctx.enter_context(tc.tile_pool(name="expp", bufs=5))
        smp = ctx.enter_context(tc.tile_pool(name="smalls", bufs=2))
        outp = ctx.enter_context(tc.tile_pool(name="outp", bufs=2))

        if nbm:
            bmask_sb = per.tile([128, nbm], MM_DT, tag="bmask")
            nc.sync.dma_start(out=bmask_sb,
                              in_=bmask_d.ap().rearrange("n p -> p n"))
            bidx = {(j, r0, r1): i for i, (j, r0, r1) in enumerate(bpairs)}

        # ---------------- phase bodies (emitted interleaved below) --------
        def emit_A(ss):
            """QKV + RoPE + transposes for s-superchunk ss."""
            xt = xts[ss]
            if ss == 1:
                for h in range(HPC):
                    nc.sync.dma_start(out=wp_sb[h],
                                      in_=wpT_d[h * HD:(h + 1) * HD, :])
            if ss > 0:
                nc.sync.dma_start(
                    out=xt,
                    in_=xblk_d[ss].rearrange("sub p dp c n -> p sub dp c n"))
            for h in range(HPC):
                # zero pad + ones column for the softmax-denominator trick
                nc.gpsimd.memset(v_sb[h][ss][:, :, HD:VEXT - 1], 0.0)
                nc.gpsimd.memset(v_sb[h][ss][:, :, VEXT - 1:VEXT], 1.0)
            # bf16 transposes packed two tensor-heads per PSUM bank
            tp2 = [psp.tile([HD, 2, 512], MM_DT, tag="tp", bufs=2, name="tpps")
                   for _ in range(HPC)]
            nh = 2 * HPC  # 4 roped qk tensor-heads
            for sub in range(4):
                qp = psp.tile([128, ODIM], F32, tag="qp", bufs=1, name="qkvps")
                for d in range(10):
                    nc.tensor.matmul(
                        qp[:, :],
                        lhsT=xt[:, sub, d // 2, d % 2, :],
                        rhs=wblk_sb[:, d, :], start=(d == 0), stop=False)
                nc.tensor.matmul(qp[:, :], lhsT=ones_sb[:, :],
                                 rhs=bqkv_sb[:, :], start=False, stop=True)

                # single PSUM->SBUF evacuation; RoPE and the v copies then
                # run from bf16 SBUF (2x DVE mode / Pool-eligible)
                qsb = qkvp.tile([128, ODIM], MM_DT, tag="qkv", name="qsb")
                with nc.allow_low_precision("bf16 evac"):
                    nc.scalar.copy(qsb, qp)

                # RoPE over q,k: out = t*cos + halfswap(t)*sinsgn
                m1 = ropep.tile([128, QKDIM], MM_DT, tag="m1")
                m2 = ropep.tile([128, QKDIM], MM_DT, tag="m2")
                qk_h = qsb[:, 0:QKDIM].rearrange("p (h d) -> p h d", h=nh)
                cos_b = cos_sb[:, ss, sub:sub + 1, :].to_broadcast(
                    [128, nh, HD])
                nc.vector.tensor_mul(
                    m1.rearrange("p (h d) -> p h d", h=nh), qk_h, cos_b)
                swap = qsb[:, 0:QKDIM].rearrange(
                    "p (h x d) -> p h x d", h=nh, x=2)[:, :, ::-1, :]
                sin_b = sin_sb[:, ss, sub:sub + 1, :].rearrange(
                    "p t (x d) -> p (t x) d", x=2)[:, None, :, :] \
                    .to_broadcast([128, nh, 2, HD // 2])
                nc.vector.tensor_mul(
                    m2.rearrange("p (h x d) -> p h x d", h=nh, x=2),
                    swap, sin_b)
                ro = qkrop.tile([128, QKDIM], MM_DT, tag="qkro")
                nc.vector.tensor_add(ro, m1, m2)

                # v natural copies are SBUF->SBUF now: Pool engine
                for h in range(HPC):
                    nc.gpsimd.tensor_copy(
                        v_sb[h][ss][:, sub, 0:HD],
                        qsb[:, QKDIM + HD * h:QKDIM + HD * (h + 1)])

                # transpose roped q,k to [hd, s]
                for j in range(2 * HPC):
                    nc.tensor.transpose(
                        tp2[j // 2][:, j % 2, 128 * sub:128 * (sub + 1)],
                        ro[:, HD * j:HD * (j + 1)], ident_sb)
            for j in range(2 * HPC):
                with nc.allow_low_precision("bf16 matmul inputs"):
                    nc.vector.tensor_copy(
                        qkT[j][:, 512 * ss:512 * (ss + 1)],
                        tp2[j // 2][:, j % 2, :])

        # stage-2 (normalize) closures deferred one group so the PE has the
        # next group's score/attn matmuls to chew on while the DVE computes
        # the reciprocal of this group's denominator row.
        pending_B: list = []

        def flush_B(keep: int = 0):
            while len(pending_B) > keep:
                pending_B.pop(0)()

        def emit_B(seg):
            """block-diagonal attention for one segment (both heads)."""
            s0, s1 = seg
            jb0, jb1 = s0 // 128, -(-s1 // 128)
            # q-chunks of <=512 (one fp32 PSUM bank per score matmul)
            qchunks = []
            g = s0
            while g < s1:
                q1 = min(s1, (g - g % 2) + 512)  # even-widened span <= 512
                qchunks.append((g, q1))
                g = q1
            for q0, q1 in qchunks:
                # widen to even alignment for bf16 matmul moving dims
                qa0 = q0 - (q0 % 2)
                qa1 = min(S, q1 + (q1 % 2))
                qna, off, qn = qa1 - qa0, q0 - qa0, q1 - q0
                for h in range(HPC):
                    ap_ = psp.tile([VEXT, 512], F32, tag="ps", name="attps")
                    blocks = list(range(jb0, jb1))
                    # pairs of k-blocks; a narrow (<=256) pair shares one
                    # PSUM bank and gets a single merged exp
                    for g0 in range(0, len(blocks), 2):
                        grp = blocks[g0:g0 + 2]
                        merged = len(grp) == 2 and qna <= 256
                        if merged:
                            sc = psp.tile([128, 2, 256], F32, tag="ps",
                                          name="scps")
                            scs = [sc[:, i, :qna] for i in range(2)]
                        else:
                            scs = [psp.tile([128, 512], F32, tag="ps",
                                            name="scps")[:, :qna]
                                   for _ in grp]
                        for i, j in enumerate(grp):
                            nc.tensor.matmul(
                                scs[i],
                                lhsT=qkT[HPC + h][:, 128 * j:128 * (j + 1)],
                                rhs=qkT[h][:, qa0:qa1],
                                start=True, stop=True)
                        if merged:
                            ex = expp.tile([128, 2, 256], MM_DT, tag="expp")
                            exs = [ex[:, i, :qna] for i in range(2)]
                            nc.scalar.activation(
                                ex[:, :, :qna], sc[:, :, :qna],
                                mybir.ActivationFunctionType.Exp)
                        else:
                            exs = []
                            for i in range(len(grp)):
                                ex = expp.tile([128, 512], MM_DT, tag="expp")
                                nc.scalar.activation(
                                    ex[:, :qna], scs[i],
                                    mybir.ActivationFunctionType.Exp)
                                exs.append(ex[:, :qna])
                        for i, j in enumerate(grp):
                            r0, r1 = max(0, s0 - 128 * j), min(128, s1 - 128 * j)
                            if r0 > 0 or r1 < 128:
                                # zero out-of-segment rows of this block
                                mi = bidx[(j, r0, r1)]
                                with nc.allow_low_precision("bf16 inputs"):
                                    nc.vector.tensor_mul(
                                        exs[i], exs[i],
                                        bmask_sb[:, mi:mi + 1]
                                        .to_broadcast([128, qna]))
                        for i, j in enumerate(grp):
                            nc.tensor.matmul(
                                ap_[:, :qna],
                                lhsT=v_sb[h][j // 4][:, j % 4, :],
                                rhs=exs[i],
                                start=(j == blocks[0]),
                                stop=(j == blocks[-1]))

                    def stage2(ap_=ap_, h=h, q0=q0, q1=q1, qna=qna, off=off,
                               qn=qn):
                        den = smp.tile([1, 512], MM_DT, tag="den", name="den")
                        with nc.allow_low_precision("bf16 matmul inputs"):
                            nc.vector.tensor_copy(den[:, :qna],
                                                  ap_[96:97, :qna])
                        bc = psp.tile([HD, 512], F32, tag="ps", name="bcps")
                        nc.tensor.matmul(bc[:, :qna], lhsT=ones_sb[:, 0:HD],
                                         rhs=den[:, :qna],
                                         start=True, stop=True)
                        rec = smp.tile([HD, 512], MM_DT, tag="rec", name="rec")
                        with nc.allow_low_precision("bf16 matmul inputs"):
                            nc.vector.reciprocal(rec[:, :qna], bc[:, :qna])
                            nc.vector.tensor_mul(att_o[h][:, q0:q1],
                                                 ap_[0:HD, off:off + qn],
                                                 rec[:, off:off + qn])

                    pending_B.append(stage2)
                    flush_B(keep=1)

        def emit_C(sc_):
            """projection for output s-superchunk sc_.
            b_proj is added host-side after the cross-core partial sum."""
            for mh in range(2):
                ob = outp.tile([128, 5, 512], MM_DT, tag="outp")
                for mm_ in range(5):
                    m = 5 * mh + mm_
                    pp = psp.tile([128, 512], F32, tag="ps", name="prps")
                    for h in range(HPC):
                        nc.tensor.matmul(
                            pp[:, :],
                            lhsT=wp_sb[h][:, 128 * m:128 * (m + 1)],
                            rhs=att_o[h][:, 512 * sc_:512 * (sc_ + 1)],
                            start=(h == 0), stop=(h == HPC - 1))
                    with nc.allow_low_precision("bf16 output"):
                        if mm_ % 2 == 0:
                            nc.vector.tensor_copy(ob[:, mm_, :], pp)
                        else:
                            nc.scalar.copy(ob[:, mm_, :], pp)
                if sc_ == NSS - 1:
                    # final superchunk: small SP-issued pieces so the tail
                    # transfer after the last copy is minimal
                    for p0, p1 in ((0, 2), (2, 4), (4, 5)):
                        nc.sync.dma_start(out=outb_d[sc_, mh, :, p0:p1],
                                          in_=ob[:, p0:p1, :])
                else:
                    nc.gpsimd.dma_start(out=outb_d[sc_, mh, :, 0:3],
                                        in_=ob[:, 0:3, :])
                    nc.gpsimd.dma_start(out=outb_d[sc_, mh, :, 3:5],
                                        in_=ob[:, 3:5, :])

        # ---- interleaved driver: emit B as soon as its span is produced,
        # ---- C as soon as all segments covering its chunk are attended.
        segs_left = sorted(segments, key=lambda s: s[1])
        c_next = 0

        def process(done_to):
            nonlocal c_next
            while segs_left and segs_left[0][1] <= done_to:
                emit_B(segs_left.pop(0))
            covered = min((s0 for (s0, s1) in segs_left), default=S)
            if c_next < NSS and 512 * (c_next + 1) <= covered:
                flush_B()
                while c_next < NSS and 512 * (c_next + 1) <= covered:
                    emit_C(c_next)
                    c_next += 1

        for ss in range(NSS):
            emit_A(ss)
            if ss >= 1:
                process(512 * ss)
        process(S)
        assert not segs_left
        flush_B()
        while c_next < NSS:
            emit_C(c_next)
            c_next += 1

    nc.compile()
    return nc


def _prep_inputs(x, cu_seqlens, rotary_pos_emb, w_qkv, b_qkv, w_proj, b_proj):
    """Host-side shard prep. Returns per-core input dicts."""
    import ml_dtypes
    bf16 = ml_dtypes.bfloat16
    scale = np.float32(1.0 / np.sqrt(np.float32(HD)))
    xT = np.asarray(x, np.float32).T
    w_qkv = np.asarray(w_qkv, np.float32)
    b_qkv = np.asarray(b_qkv, np.float32)
    w_proj = np.asarray(w_proj, np.float32)
    rot = np.asarray(rotary_pos_emb, np.float32)

    cosw = np.concatenate([np.cos(rot), np.cos(rot)], axis=1).astype(np.float32)
    sinw = np.concatenate([-np.sin(rot), np.sin(rot)], axis=1).astype(np.float32)
    # blocked layouts so every device DMA reads one dense contiguous region:
    # xblk[ss,sub,p,dp,c,n] = xT[256dp+128c+p, 512ss+128sub+n]
    xblk = np.ascontiguousarray(
        xT.reshape(5, 2, 128, 6, 4, 128).transpose(3, 4, 2, 0, 1, 5)) \
        .astype(bf16)
    # cosb[p,ss,t,d] = cosw[512ss+128t+p, d]
    cosb = np.ascontiguousarray(
        cosw.reshape(6, 4, 128, HD).transpose(2, 0, 1, 3)).astype(bf16)
    sinb = np.ascontiguousarray(
        sinw.reshape(6, 4, 128, HD).transpose(2, 0, 1, 3)).astype(bf16)
    ident = np.eye(128, dtype=np.float32).astype(bf16)
    onesrow = np.ones((1, 128), dtype=np.float32).astype(bf16)
    vpad = np.zeros(17, dtype=np.float32)
    vpad[16] = 1.0
    vpad = vpad.astype(bf16)
    segments = _segments_from_cu(cu_seqlens)
    bmask_rows = []
    for (j, r0, r1) in _bpairs(segments):
        row = np.zeros(128, dtype=np.float32)
        row[r0:r1] = 1.0
        bmask_rows.append(row)
    bmask = (np.stack(bmask_rows).astype(bf16) if bmask_rows else None)

    in_maps = []
    for c in range(NCORES):
        heads = [HPC * c + i for i in range(HPC)]
        idx = []
        for base in (0, DIM, 2 * DIM):           # q, k, v row blocks
            for h in heads:
                idx.extend(range(base + h * HD, base + (h + 1) * HD))
        w_c = w_qkv[idx, :].copy()
        b_c = b_qkv[idx].copy()
        w_c[:QKDIM // 2] *= scale                # scale q by 1/sqrt(HD)
        b_c[:QKDIM // 2] *= scale
        # wblk[p, d, o] = w_c.T[128d+p, o]
        wblk = np.ascontiguousarray(
            w_c.T.reshape(10, 128, ODIM).transpose(1, 0, 2)).astype(bf16)
        cdims = []
        for h in heads:
            cdims.extend(range(h * HD, (h + 1) * HD))
        wpT = np.ascontiguousarray(w_proj[:, cdims].T).astype(bf16)
        in_maps.append({
            "xblk": xblk,
            "wblk": wblk,
            "bqkv": np.ascontiguousarray(b_c[None, :]).astype(bf16),
            "cosb": cosb,
            "sinb": sinb,
            "wpT": wpT,
            "ident": ident,
            "onesrow": onesrow,
            "vpad": vpad,
        })
        if bmask is not None:
            in_maps[-1]["bmask"] = bmask
    return in_maps


def run(inputs: dict, trace: bool = False):
    """Build (cached), run on 8 cores, return (out [S, DIM] fp32, results)."""
    segments = _segments_from_cu(inputs["cu_seqlens"])
    key = (segments, str(MM_DT))
    if key not in _CACHE:
        _CACHE[key] = _build(segments)
    nc = _CACHE[key]
    in_maps = _prep_inputs(
        inputs["x"], inputs["cu_seqlens"], inputs["rotary_pos_emb"],
        inputs["w_qkv"], inputs["b_qkv"], inputs["w_proj"], inputs["b_proj"])
    res = run_bass_kernel_spmd(nc, in_maps, core_ids=list(range(NCORES)),
                               trace=trace)
    acc = np.zeros((DIM, S), np.float64)
    for r in res.results:
        # blocked [sc, mh, p, c, n] -> [dim = 640*mh+128*c+p, s = 512*sc+n]
        acc += r["outb"].astype(np.float64).transpose(1, 3, 2, 0, 4) \
            .reshape(DIM, S)
    acc += np.asarray(inputs["b_proj"], np.float64)[:, None]
    out = np.ascontiguousarray(acc.T.astype(np.float32))
    return out, res


def kernel(**inputs) -> np.ndarray:
    out, _ = run(inputs, trace=False)
    return out


# revision 38
# speedup vs baseline: 2762.9566x; 1.0147x over previous
"""Trainium2 Bass kernel for nn_Attention_72541997629647 (sparse varlen attention).

Computation (see problem reference):
  qkv = x @ w_qkv.T + b_qkv ; NeoX RoPE on q,k ; block-diagonal softmax
  attention from cu_seqlens segments ; out = (attn @ v) @ w_proj.T + b_proj

Sharding: tensor-parallel over heads. 16 heads / 8 cores = 2 heads per core.
Each core computes q/k/v for its 2 heads, runs block-diagonal attention, and
produces a partial projection output (full [DIM, S], transposed); the host
sums the 8 partials and adds b_proj, so the result is exact.

All matmul inputs and all DMA payloads are bf16 (PSUM accumulation stays
fp32); this halves HBM traffic and DVE elementwise cost vs fp32 while staying
well inside the 2e-2 tolerance.

Device dataflow per core:
  A) QKV: out_nat[s, 480] = xT-chunks.T @ w_chunks (+ bias via ones-row
     matmul); one ACT copy evacuates each PSUM group to bf16 SBUF; RoPE runs
     on bf16 SBUF (2x DVE mode, half-swap via negative-step AP, sign folded
     into the host-built sin table); q,k PE-transposed (bf16, two heads per
     PSUM bank) into persistent [hd, S] tiles, evacuated by DVE; v copied
     natural by the Pool engine, with a memset-built ones column at col 96
     (softmax-denominator trick).
  B) per (head, segment, q-chunk<=512): scoresT[k,q] = kT-block.T @ qT; exp
     on ACT -> bf16 SBUF (narrow chunks share a PSUM bank pairwise with one
     merged exp); attT[97, q] += v_ext.T @ exp accumulated over k-blocks;
     row 96 is the denominator. normalize = den copy + ones-matmul
     partition-broadcast + reciprocal + multiply, software-pipelined one
     group behind the matmuls so the PE never waits on the DVE reciprocal.
     PSUM slots are tag-partitioned (qkv accum / transpose banks / the rest)
     so phase B never FIFO-blocks on phase A's banks.
  C) proj: outT[dim, s] += wpT-head.T @ attT-head ; PSUM->SBUF bf16 copies
     split across ACT/DVE; output DMAd as dense bf16 blocks from the Pool
     engine (SWDGE; small SP-issued pieces for the final superchunk so the
     tail transfer is minimal) and unscrambled on the host. b_proj is added
     host-side after the cross-core partial sum.
"""

import os
import sys

for _p in ("/opt/trn_rl_repo", "/root/.axon_site/_ro/trn_rl_repo"):
    if os.path.isdir(_p) and _p not in sys.path:
        sys.path.insert(0, _p)

import numpy as np

import concourse.bacc as bacc
import concourse.bass as bass
import concourse.mybir as mybir
import concourse.tile as tile
from concourse.bass_utils import run_bass_kernel_spmd
from contextlib import ExitStack

S = 3072
DIM = 1280
H = 16
HD = 80
NCORES = 8
HPC = H // NCORES          # heads per core = 2
QKDIM = 2 * HPC * HD       # 320 (q+k outdims per core)
ODIM = 3 * HPC * HD        # 480 (qkv outdims per core)
CDIM = HPC * HD            # 160 (attn channels per core)
VEXT = 97                  # v extended: 80 v-dims, 16 pad, ones col at 96

F32 = mybir.dt.float32
BF16 = mybir.dt.bfloat16
MM_DT = BF16

_CACHE: dict = {}


def _segments_from_cu(cu_seqlens: np.ndarray) -> tuple:
    """Contiguous runs of equal segment id, exactly as the reference's
    searchsorted-based mask defines them."""
    cu = np.asarray(cu_seqlens).astype(np.int64)
    seg = np.searchsorted(cu, np.arange(S), side="right") - 1
    change = np.nonzero(np.diff(seg))[0]
    starts = np.concatenate([[0], change + 1])
    ends = np.concatenate([change + 1, [S]])
    return tuple((int(a), int(b)) for a, b in zip(starts, ends))


def _bpairs(segments):
    """Boundary (block, r0, r1) triples needing a 0/1 row mask, in traversal
    order shared by host prep and device emission."""
    out = []
    for (s0, s1) in segments:
        for j in range(s0 // 128, -(-s1 // 128)):
            r0, r1 = max(0, s0 - 128 * j), min(128, s1 - 128 * j)
            if r0 > 0 or r1 < 128:
                out.append((j, r0, r1))
    return out


def _build(segments, loop_n: int = 1) -> "bacc.Bacc":
    nc = bacc.Bacc("TRN2", target_bir_lowering=False, debug=False,
                   num_devices=NCORES)

    NSS = S // 512  # 6 s-superchunks

    xblk_d = nc.dram_tensor("xblk", [NSS, 4, 128, 5, 2, 128], MM_DT,
                            kind="ExternalInput")
    wblk_d = nc.dram_tensor("wblk", [128, 10, ODIM], MM_DT,
                            kind="ExternalInput")
    bqkv_d = nc.dram_tensor("bqkv", [1, ODIM], MM_DT, kind="ExternalInput")
    cosb_d = nc.dram_tensor("cosb", [128, NSS, 4, HD], MM_DT,
                            kind="ExternalInput")
    sinb_d = nc.dram_tensor("sinb", [128, NSS, 4, HD], MM_DT,
                            kind="ExternalInput")
    wpT_d = nc.dram_tensor("wpT", [CDIM, DIM], MM_DT, kind="ExternalInput")
    ident_d = nc.dram_tensor("ident", [128, 128], MM_DT, kind="ExternalInput")
    ones_d = nc.dram_tensor("onesrow", [1, 128], MM_DT, kind="ExternalInput")
    vpad_d = nc.dram_tensor("vpad", [17], MM_DT, kind="ExternalInput")
    bpairs = _bpairs(segments)
    nbm = len(bpairs)
    bmask_d = (nc.dram_tensor("bmask", [nbm, 128], MM_DT, kind="ExternalInput")
               if nbm else None)
    outb_d = nc.dram_tensor("outb", [NSS, 2, 128, 5, 512], MM_DT,
                            kind="ExternalOutput")

    with tile.TileContext(nc) as tc, ExitStack() as ctx:
        if loop_n > 1:  # benchmarking only: repeat the whole body on-device
            ctx.enter_context(tc.For_i(0, loop_n, 1))
        per = ctx.enter_context(tc.tile_pool(name="persist", bufs=1))

        # qkv weights first (every matmul needs them), then x superchunk 0;
        # everything else is needed later and queues behind them on the
        # shared DMA engines.
        wblk_sb = per.tile([128, 10, ODIM], MM_DT, tag="wblk")
        xtp = ctx.enter_context(tc.tile_pool(name="xt", bufs=6))
        xts = []
        for ss in range(NSS):
            xts.append(xtp.tile([128, 4, 5, 2, 128], MM_DT, tag="xt",
                                name="xt"))
        nc.sync.dma_start(out=wblk_sb[:, 0:5, :], in_=wblk_d[:, 0:5, :])
        nc.sync.dma_start(out=xts[0][:, 0], in_=xblk_d[0, 0])
        nc.sync.dma_start(out=wblk_sb[:, 5:10, :], in_=wblk_d[:, 5:10, :])
        nc.sync.dma_start(out=xts[0][:, 1], in_=xblk_d[0, 1])
        bqkv_sb = per.tile([1, ODIM], MM_DT, tag="bqkv")
        nc.sync.dma_start(out=bqkv_sb, in_=bqkv_d[:, :])
        ones_sb = per.tile([1, 128], MM_DT, tag="ones")
        nc.sync.dma_start(out=ones_sb, in_=ones_d[:, :])
        cos_sb = per.tile([128, NSS, 4, HD], MM_DT, tag="cos")
        nc.sync.dma_start(out=cos_sb, in_=cosb_d.ap())
        sin_sb = per.tile([128, NSS, 4, HD], MM_DT, tag="sin")
        nc.sync.dma_start(out=sin_sb, in_=sinb_d.ap())
        for sub in range(2, 4):
            nc.sync.dma_start(out=xts[0][:, sub], in_=xblk_d[0, sub])
        ident_sb = per.tile([128, 128], MM_DT, tag="ident")
        nc.sync.dma_start(out=ident_sb, in_=ident_d[:, :])
        wp_sb = [per.tile([HD, DIM], MM_DT, tag=f"wp{h}", name=f"wp{h}")
                 for h in range(HPC)]

        # persistent transposed q,k [hd, S] and attention output [hd, S];
        # v natural per 512-superchunk with the ones column for denominators
        qkT = [per.tile([HD, S], MM_DT, tag=f"qkT{j}", name=f"qkT{j}")
               for j in range(2 * HPC)]
        att_o = [per.tile([HD, S], MM_DT, tag=f"atto{h}", name=f"atto{h}")
                 for h in range(HPC)]
        v_sb = [[per.tile([128, 4, VEXT], MM_DT, tag=f"v{h}_{ss}",
                          name=f"v{h}_{ss}") for ss in range(NSS)]
                for h in range(HPC)]

        # one shared PSUM pool (8 bank-sized slots shared by every phase so
        # the scheduler can overlap A/B/C), plus top-level SBUF pools
        psp = ctx.enter_context(tc.tile_pool(name="ps", bufs=5, space="PSUM"))
        qkvp = ctx.enter_context(tc.tile_pool(name="qkvsb", bufs=3))
        ropep = ctx.enter_context(tc.tile_pool(name="ropet", bufs=2))
        qkrop = ctx.enter_context(tc.tile_pool(name="qkro", bufs=3))
        expp = ctx.enter_context(tc.tile_pool(name="expp", bufs=5))
        smp = ctx.enter_context(tc.tile_pool(name="smalls", bufs=2))
        outp = ctx.enter_context(tc.tile_pool(name="outp", bufs=2))

        if nbm:
            bmask_sb = per.tile([128, nbm], MM_DT, tag="bmask")
            nc.sync.dma_start(out=bmask_sb,
                              in_=bmask_d.ap().rearrange("n p -> p n"))
            bidx = {(j, r0, r1): i for i, (j, r0, r1) in enumerate(bpairs)}

        # ---------------- phase bodies (emitted interleaved below) --------
        def emit_A(ss):
            """QKV + RoPE + transposes for s-superchunk ss."""
            xt = xts[ss]
            if ss > 0:
                nc.sync.dma_start(
                    out=xt,
                    in_=xblk_d[ss].rearrange("sub p dp c n -> p sub dp c n"))
            if ss == 1:
                for h in range(HPC):
                    nc.sync.dma_start(out=wp_sb[h],
                                      in_=wpT_d[h * HD:(h + 1) * HD, :])
            for h in range(HPC):
                # zero pad + ones column for the softmax-denominator trick
                nc.gpsimd.memset(v_sb[h][ss][:, :, HD:VEXT - 1], 0.0)
                nc.gpsimd.memset(v_sb[h][ss][:, :, VEXT - 1:VEXT], 1.0)
            # bf16 transposes packed two tensor-heads per PSUM bank
            tp2 = [psp.tile([HD, 2, 512], MM_DT, tag="tp", bufs=2, name="tpps")
                   for _ in range(HPC)]
            nh = 2 * HPC  # 4 roped qk tensor-heads
            for sub in range(4):
                qp = psp.tile([128, ODIM], F32, tag="qp", bufs=1, name="qkvps")
                for d in range(10):
                    nc.tensor.matmul(
                        qp[:, :],
                        lhsT=xt[:, sub, d // 2, d % 2, :],
                        rhs=wblk_sb[:, d, :], start=(d == 0), stop=False)
                nc.tensor.matmul(qp[:, :], lhsT=ones_sb[:, :],
                                 rhs=bqkv_sb[:, :], start=False, stop=True)

                # single PSUM->SBUF evacuation; RoPE and the v copies then
                # run from bf16 SBUF (2x DVE mode / Pool-eligible)
                qsb = qkvp.tile([128, ODIM], MM_DT, tag="qkv", name="qsb")
                with nc.allow_low_precision("bf16 evac"):
                    nc.scalar.copy(qsb, qp)

                # RoPE over q,k: out = t*cos + halfswap(t)*sinsgn
                m1 = ropep.tile([128, QKDIM], MM_DT, tag="m1")
                m2 = ropep.tile([128, QKDIM], MM_DT, tag="m2")
                qk_h = qsb[:, 0:QKDIM].rearrange("p (h d) -> p h d", h=nh)
                cos_b = cos_sb[:, ss, sub:sub + 1, :].to_broadcast(
                    [128, nh, HD])
                nc.vector.tensor_mul(
                    m1.rearrange("p (h d) -> p h d", h=nh), qk_h, cos_b)
                swap = qsb[:, 0:QKDIM].rearrange(
                    "p (h x d) -> p h x d", h=nh, x=2)[:, :, ::-1, :]
                sin_b = sin_sb[:, ss, sub:sub + 1, :].rearrange(
                    "p t (x d) -> p (t x) d", x=2)[:, None, :, :] \
                    .to_broadcast([128, nh, 2, HD // 2])
                nc.vector.tensor_mul(
                    m2.rearrange("p (h x d) -> p h x d", h=nh, x=2),
                    swap, sin_b)
                ro = qkrop.tile([128, QKDIM], MM_DT, tag="qkro")
                nc.vector.tensor_add(ro, m1, m2)

                # v natural copies are SBUF->SBUF now: Pool engine
                for h in range(HPC):
                    nc.gpsimd.tensor_copy(
                        v_sb[h][ss][:, sub, 0:HD],
                        qsb[:, QKDIM + HD * h:QKDIM + HD * (h + 1)])

                # transpose roped q,k to [hd, s]
                for j in range(2 * HPC):
                    nc.tensor.transpose(
                        tp2[j // 2][:, j % 2, 128 * sub:128 * (sub + 1)],
                        ro[:, HD * j:HD * (j + 1)], ident_sb)
            for j in range(2 * HPC):
                with nc.allow_low_precision("bf16 matmul inputs"):
                    nc.vector.tensor_copy(
                        qkT[j][:, 512 * ss:512 * (ss + 1)],
                        tp2[j // 2][:, j % 2, :])

        # stage-2 (normalize) closures deferred one group so the PE has the
        # next group's score/attn matmuls to chew on while the DVE computes
        # the reciprocal of this group's denominator row.
        pending_B: list = []

        def flush_B(keep: int = 0):
            while len(pending_B) > keep:
                pending_B.pop(0)()

        def emit_B(seg, on_progress=None):
            """block-diagonal attention for one segment (both heads)."""
            s0, s1 = seg
            jb0, jb1 = s0 // 128, -(-s1 // 128)
            # q-chunks of <=512 (one fp32 PSUM bank per score matmul)
            qchunks = []
            g = s0
            while g < s1:
                q1 = min(s1, (g - g % 2) + 512)  # even-widened span <= 512
                qchunks.append((g, q1))
                g = q1
            for q0, q1 in qchunks:
                # widen to even alignment for bf16 matmul moving dims
                qa0 = q0 - (q0 % 2)
                qa1 = min(S, q1 + (q1 % 2))
                qna, off, qn = qa1 - qa0, q0 - qa0, q1 - q0
                for h in range(HPC):
                    ap_ = psp.tile([VEXT, 512], F32, tag="ps", name="attps")
                    blocks = list(range(jb0, jb1))
                    # pairs of k-blocks; a narrow (<=256) pair shares one
                    # PSUM bank and gets a single merged exp
                    for g0 in range(0, len(blocks), 2):
                        grp = blocks[g0:g0 + 2]
                        merged = len(grp) == 2 and qna <= 256
                        if merged:
                            sc = psp.tile([128, 2, 256], F32, tag="ps",
                                          name="scps")
                            scs = [sc[:, i, :qna] for i in range(2)]
                        else:
                            scs = [psp.tile([128, 512], F32, tag="ps",
                                            name="scps")[:, :qna]
                                   for _ in grp]
                        for i, j in enumerate(grp):
                            nc.tensor.matmul(
                                scs[i],
                                lhsT=qkT[HPC + h][:, 128 * j:128 * (j + 1)],
                                rhs=qkT[h][:, qa0:qa1],
                                start=True, stop=True)
                        if merged:
                            ex = expp.tile([128, 2, 256], MM_DT, tag="expp")
                            exs = [ex[:, i, :qna] for i in range(2)]
                            nc.scalar.activation(
                                ex[:, :, :qna], sc[:, :, :qna],
                                mybir.ActivationFunctionType.Exp)
                        else:
                            exs = []
                            for i in range(len(grp)):
                                ex = expp.tile([128, 512], MM_DT, tag="expp")
                                nc.scalar.activation(
                                    ex[:, :qna], scs[i],
                                    mybir.ActivationFunctionType.Exp)
                                exs.append(ex[:, :qna])
                        for i, j in enumerate(grp):
                            r0, r1 = max(0, s0 - 128 * j), min(128, s1 - 128 * j)
                            if r0 > 0 or r1 < 128:
                                # zero out-of-segment rows of this block
                                mi = bidx[(j, r0, r1)]
                                with nc.allow_low_precision("bf16 inputs"):
                                    nc.vector.tensor_mul(
                                        exs[i], exs[i],
                                        bmask_sb[:, mi:mi + 1]
                                        .to_broadcast([128, qna]))
                        for i, j in enumerate(grp):
                            nc.tensor.matmul(
                                ap_[:, :qna],
                                lhsT=v_sb[h][j // 4][:, j % 4, :],
                                rhs=exs[i],
                                start=(j == blocks[0]),
                                stop=(j == blocks[-1]))

                    def stage2(ap_=ap_, h=h, q0=q0, q1=q1, qna=qna, off=off,
                               qn=qn):
                        den = smp.tile([1, 512], MM_DT, tag="den", name="den")
                        with nc.allow_low_precision("bf16 matmul inputs"):
                            nc.vector.tensor_copy(den[:, :qna],
                                                  ap_[96:97, :qna])
                        bc = psp.tile([HD, 512], F32, tag="ps", name="bcps")
                        nc.tensor.matmul(bc[:, :qna], lhsT=ones_sb[:, 0:HD],
                                         rhs=den[:, :qna],
                                         start=True, stop=True)
                        rec = smp.tile([HD, 512], MM_DT, tag="rec", name="rec")
                        with nc.allow_low_precision("bf16 matmul inputs"):
                            nc.vector.reciprocal(rec[:, :qna], bc[:, :qna])
                            nc.vector.tensor_mul(att_o[h][:, q0:q1],
                                                 ap_[0:HD, off:off + qn],
                                                 rec[:, off:off + qn])

                    pending_B.append(stage2)
                    flush_B(keep=1)
                if on_progress:
                    on_progress(q1)

        def emit_C(sc_):
            """projection for output s-superchunk sc_.
            b_proj is added host-side after the cross-core partial sum."""
            for mh in range(2):
                ob = outp.tile([128, 5, 512], MM_DT, tag="outp")
                for mm_ in range(5):
                    m = 5 * mh + mm_
                    pp = psp.tile([128, 512], F32, tag="ps", name="prps")
                    for h in range(HPC):
                        nc.tensor.matmul(
                            pp[:, :],
                            lhsT=wp_sb[h][:, 128 * m:128 * (m + 1)],
                            rhs=att_o[h][:, 512 * sc_:512 * (sc_ + 1)],
                            start=(h == 0), stop=(h == HPC - 1))
                    with nc.allow_low_precision("bf16 output"):
                        if mm_ % 2 == 0:
                            nc.vector.tensor_copy(ob[:, mm_, :], pp)
                        else:
                            nc.scalar.copy(ob[:, mm_, :], pp)
                if sc_ == NSS - 1:
                    # final superchunk: small SP-issued pieces so the tail
                    # transfer after the last copy is minimal
                    for p0, p1 in ((0, 2), (2, 4), (4, 5)):
                        nc.sync.dma_start(out=outb_d[sc_, mh, :, p0:p1],
                                          in_=ob[:, p0:p1, :])
                else:
                    nc.gpsimd.dma_start(out=outb_d[sc_, mh, :, 0:3],
                                        in_=ob[:, 0:3, :])
                    nc.gpsimd.dma_start(out=outb_d[sc_, mh, :, 3:5],
                                        in_=ob[:, 3:5, :])

        # ---- interleaved driver: emit B as soon as its span is produced,
        # ---- C as soon as all segments covering its chunk are attended.
        segs_left = sorted(segments, key=lambda s: s[1])
        c_next = 0

        def process(done_to, frontier_cb=None):
            nonlocal c_next
            while segs_left and segs_left[0][1] <= done_to:
                emit_B(segs_left.pop(0), on_progress=frontier_cb)
            covered = min((s0 for (s0, s1) in segs_left), default=S)
            if c_next < NSS and 512 * (c_next + 1) <= covered:
                flush_B()
                while c_next < NSS and 512 * (c_next + 1) <= covered:
                    emit_C(c_next)
                    c_next += 1

        for ss in range(NSS):
            emit_A(ss)
            if ss >= 1:
                process(512 * ss)
        process(S)
        assert not segs_left
        flush_B()
        while c_next < NSS:
            emit_C(c_next)
            c_next += 1

    nc.compile()
    return nc


def _prep_inputs(x, cu_seqlens, rotary_pos_emb, w_qkv, b_qkv, w_proj, b_proj):
    """Host-side shard prep. Returns per-core input dicts."""
    import ml_dtypes
    bf16 = ml_dtypes.bfloat16
    scale = np.float32(1.0 / np.sqrt(np.float32(HD)))
    xT = np.asarray(x, np.float32).T
    w_qkv = np.asarray(w_qkv, np.float32)
    b_qkv = np.asarray(b_qkv, np.float32)
    w_proj = np.asarray(w_proj, np.float32)
    rot = np.asarray(rotary_pos_emb, np.float32)

    cosw = np.concatenate([np.cos(rot), np.cos(rot)], axis=1).astype(np.float32)
    sinw = np.concatenate([-np.sin(rot), np.sin(rot)], axis=1).astype(np.float32)
    # blocked layouts so every device DMA reads one dense contiguous region:
    # xblk[ss,sub,p,dp,c,n] = xT[256dp+128c+p, 512ss+128sub+n]
    xblk = np.ascontiguousarray(
        xT.reshape(5, 2, 128, 6, 4, 128).transpose(3, 4, 2, 0, 1, 5)) \
        .astype(bf16)
    # cosb[p,ss,t,d] = cosw[512ss+128t+p, d]
    cosb = np.ascontiguousarray(
        cosw.reshape(6, 4, 128, HD).transpose(2, 0, 1, 3)).astype(bf16)
    sinb = np.ascontiguousarray(
        sinw.reshape(6, 4, 128, HD).transpose(2, 0, 1, 3)).astype(bf16)
    ident = np.eye(128, dtype=np.float32).astype(bf16)
    onesrow = np.ones((1, 128), dtype=np.float32).astype(bf16)
    vpad = np.zeros(17, dtype=np.float32)
    vpad[16] = 1.0
    vpad = vpad.astype(bf16)
    segments = _segments_from_cu(cu_seqlens)
    bmask_rows = []
    for (j, r0, r1) in _bpairs(segments):
        row = np.zeros(128, dtype=np.float32)
        row[r0:r1] = 1.0
        bmask_rows.append(row)
    bmask = (np.stack(bmask_rows).astype(bf16) if bmask_rows else None)

    in_maps = []
    for c in range(NCORES):
        heads = [HPC * c + i for i in range(HPC)]
        idx = []
        for base in (0, DIM, 2 * DIM):           # q, k, v row blocks
            for h in heads:
                idx.extend(range(base + h * HD, base + (h + 1) * HD))
        w_c = w_qkv[idx, :].copy()
        b_c = b_qkv[idx].copy()
        w_c[:QKDIM // 2] *= scale                # scale q by 1/sqrt(HD)
        b_c[:QKDIM // 2] *= scale
        # wblk[p, d, o] = w_c.T[128d+p, o]
        wblk = np.ascontiguousarray(
            w_c.T.reshape(10, 128, ODIM).transpose(1, 0, 2)).astype(bf16)
        cdims = []
        for h in heads:
            cdims.extend(range(h * HD, (h + 1) * HD))
        wpT = np.ascontiguousarray(w_proj[:, cdims].T).astype(bf16)
        in_maps.append({
            "xblk": xblk,
            "wblk": wblk,
            "bqkv": np.ascontiguousarray(b_c[None, :]).astype(bf16),
            "cosb": cosb,
            "sinb": sinb,
            "wpT": wpT,
            "ident": ident,
            "onesrow": onesrow,
            "vpad": vpad,
        })
        if bmask is not None:
            in_maps[-1]["bmask"] = bmask
    return in_maps


def run(inputs: dict, trace: bool = False):
    """Build (cached), run on 8 cores, return (out [S, DIM] fp32, results)."""
    segments = _segments_from_cu(inputs["cu_seqlens"])
    key = (segments, str(MM_DT))
    if key not in _CACHE:
        _CACHE[key] = _build(segments)
    nc = _CACHE[key]
    in_maps = _prep_inputs(
        inputs["x"], inputs["cu_seqlens"], inputs["rotary_pos_emb"],
        inputs["w_qkv"], inputs["b_qkv"], inputs["w_proj"], inputs["b_proj"])
    res = run_bass_kernel_spmd(nc, in_maps, core_ids=list(range(NCORES)),
                               trace=trace)
    acc = np.zeros((DIM, S), np.float64)
    for r in res.results:
        # blocked [sc, mh, p, c, n] -> [dim = 640*mh+128*c+p, s = 512*sc+n]
        acc += r["outb"].astype(np.float64).transpose(1, 3, 2, 0, 4) \
            .reshape(DIM, S)
    acc += np.asarray(inputs["b_proj"], np.float64)[:, None]
    out = np.ascontiguousarray(acc.T.astype(np.float32))
    return out, res


def kernel(**inputs) -> np.ndarray:
    out, _ = run(inputs, trace=False)
    return out


# revision 40
# speedup vs baseline: 2875.3247x; 1.0407x over previous
"""Trainium2 Bass kernel for nn_Attention_72541997629647 (sparse varlen attention).

Computation (see problem reference):
  qkv = x @ w_qkv.T + b_qkv ; NeoX RoPE on q,k ; block-diagonal softmax
  attention from cu_seqlens segments ; out = (attn @ v) @ w_proj.T + b_proj

Sharding: tensor-parallel over heads. 16 heads / 8 cores = 2 heads per core.
Each core computes q/k/v for its 2 heads, runs block-diagonal attention, and
produces a partial projection output (full [DIM, S], transposed); the host
sums the 8 partials and adds b_proj, so the result is exact.

All matmul inputs and all DMA payloads are bf16 (PSUM accumulation stays
fp32); this halves HBM traffic and DVE elementwise cost vs fp32 while staying
well inside the 2e-2 tolerance.

Device dataflow per core:
  A) QKV: out_nat[s, 480] = xT-chunks.T @ w_chunks (+ bias via ones-row
     matmul); one ACT copy evacuates each PSUM group to bf16 SBUF; RoPE runs
     on bf16 SBUF (2x DVE mode, half-swap via negative-step AP, sign folded
     into the host-built sin table); q,k PE-transposed (bf16, two heads per
     PSUM bank) into persistent [hd, S] tiles, evacuated by DVE; v copied
     natural by the Pool engine, with a memset-built ones column at col 96
     (softmax-denominator trick).
  B) per (head, segment, q-chunk<=512): scoresT[k,q] = kT-block.T @ qT; exp
     on ACT -> bf16 SBUF (narrow chunks share a PSUM bank pairwise with one
     merged exp); attT[97, q] += v_ext.T @ exp accumulated over k-blocks;
     row 96 is the denominator. normalize = den copy + ones-matmul
     partition-broadcast + reciprocal + multiply, software-pipelined one
     group behind the matmuls so the PE never waits on the DVE reciprocal.
     PSUM slots are tag-partitioned (qkv accum / transpose banks / the rest)
     so phase B never FIFO-blocks on phase A's banks.
  C) proj: outT[dim, s] += wpT-head.T @ attT-head ; PSUM->SBUF bf16 copies
     split across ACT/DVE; output DMAd as dense bf16 blocks from the Pool
     engine (SWDGE; small SP-issued pieces for the final superchunk so the
     tail transfer is minimal) and unscrambled on the host. b_proj is added
     host-side after the cross-core partial sum.
"""

import os
import sys

for _p in ("/opt/trn_rl_repo", "/root/.axon_site/_ro/trn_rl_repo"):
    if os.path.isdir(_p) and _p not in sys.path:
        sys.path.insert(0, _p)

import numpy as np

import concourse.bacc as bacc
import concourse.bass as bass
import concourse.mybir as mybir
import concourse.tile as tile
from concourse.bass_utils import run_bass_kernel_spmd
from contextlib import ExitStack

S = 3072
DIM = 1280
H = 16
HD = 80
NCORES = 8
HPC = H // NCORES          # heads per core = 2
QKDIM = 2 * HPC * HD       # 320 (q+k outdims per core)
ODIM = 3 * HPC * HD        # 480 (qkv outdims per core)
CDIM = HPC * HD            # 160 (attn channels per core)
VEXT = 97                  # v extended: 80 v-dims, 16 pad, ones col at 96

F32 = mybir.dt.float32
BF16 = mybir.dt.bfloat16
MM_DT = BF16

_CACHE: dict = {}


def _segments_from_cu(cu_seqlens: np.ndarray) -> tuple:
    """Contiguous runs of equal segment id, exactly as the reference's
    searchsorted-based mask defines them."""
    cu = np.asarray(cu_seqlens).astype(np.int64)
    seg = np.searchsorted(cu, np.arange(S), side="right") - 1
    change = np.nonzero(np.diff(seg))[0]
    starts = np.concatenate([[0], change + 1])
    ends = np.concatenate([change + 1, [S]])
    return tuple((int(a), int(b)) for a, b in zip(starts, ends))


def _bpairs(segments):
    """Boundary (block, r0, r1) triples needing a 0/1 row mask, in traversal
    order shared by host prep and device emission."""
    out = []
    for (s0, s1) in segments:
        for j in range(s0 // 128, -(-s1 // 128)):
            r0, r1 = max(0, s0 - 128 * j), min(128, s1 - 128 * j)
            if r0 > 0 or r1 < 128:
                out.append((j, r0, r1))
    return out


def _build(segments, loop_n: int = 1) -> "bacc.Bacc":
    nc = bacc.Bacc("TRN2", target_bir_lowering=False, debug=False,
                   num_devices=NCORES)

    NSS = S // 512  # 6 s-superchunks

    xblk_d = nc.dram_tensor("xblk", [NSS, 4, 128, 5, 2, 128], MM_DT,
                            kind="ExternalInput")
    wblk_d = nc.dram_tensor("wblk", [128, 10, ODIM], MM_DT,
                            kind="ExternalInput")
    bqkv_d = nc.dram_tensor("bqkv", [128, ODIM], MM_DT, kind="ExternalInput")
    cosb_d = nc.dram_tensor("cosb", [128, NSS, 4, HD], MM_DT,
                            kind="ExternalInput")
    sinb_d = nc.dram_tensor("sinb", [128, NSS, 4, HD], MM_DT,
                            kind="ExternalInput")
    wpT_d = nc.dram_tensor("wpT", [CDIM, DIM], MM_DT, kind="ExternalInput")
    ident_d = nc.dram_tensor("ident", [128, 128], MM_DT, kind="ExternalInput")
    ones_d = nc.dram_tensor("onesrow", [1, 128], MM_DT, kind="ExternalInput")
    vpad_d = nc.dram_tensor("vpad", [17], MM_DT, kind="ExternalInput")
    bpairs = _bpairs(segments)
    nbm = len(bpairs)
    bmask_d = (nc.dram_tensor("bmask", [nbm, 128], MM_DT, kind="ExternalInput")
               if nbm else None)
    outb_d = nc.dram_tensor("outb", [NSS, 2, 128, 5, 512], MM_DT,
                            kind="ExternalOutput")

    with tile.TileContext(nc) as tc, ExitStack() as ctx:
        if loop_n > 1:  # benchmarking only: repeat the whole body on-device
            ctx.enter_context(tc.For_i(0, loop_n, 1))
        per = ctx.enter_context(tc.tile_pool(name="persist", bufs=1))

        # qkv weights first (every matmul needs them), then x superchunk 0;
        # everything else is needed later and queues behind them on the
        # shared DMA engines.
        wblk_sb = per.tile([128, 10, ODIM], MM_DT, tag="wblk")
        xtp = ctx.enter_context(tc.tile_pool(name="xt", bufs=6))
        xts = []
        for ss in range(NSS):
            xts.append(xtp.tile([128, 4, 5, 2, 128], MM_DT, tag="xt",
                                name="xt"))
        nc.sync.dma_start(out=wblk_sb[:, 0:5, :], in_=wblk_d[:, 0:5, :])
        nc.sync.dma_start(out=xts[0][:, 0], in_=xblk_d[0, 0])
        nc.sync.dma_start(out=wblk_sb[:, 5:10, :], in_=wblk_d[:, 5:10, :])
        nc.sync.dma_start(out=xts[0][:, 1], in_=xblk_d[0, 1])
        brow_sb = per.tile([128, ODIM], MM_DT, tag="brow")
        nc.sync.dma_start(out=brow_sb, in_=bqkv_d.ap())
        ones_sb = per.tile([1, 128], MM_DT, tag="ones")
        nc.sync.dma_start(out=ones_sb, in_=ones_d[:, :])
        cos_sb = per.tile([128, NSS, 4, HD], MM_DT, tag="cos")
        nc.sync.dma_start(out=cos_sb, in_=cosb_d.ap())
        sin_sb = per.tile([128, NSS, 4, HD], MM_DT, tag="sin")
        nc.sync.dma_start(out=sin_sb, in_=sinb_d.ap())
        for sub in range(2, 4):
            nc.sync.dma_start(out=xts[0][:, sub], in_=xblk_d[0, sub])
        ident_sb = per.tile([128, 128], MM_DT, tag="ident")
        nc.sync.dma_start(out=ident_sb, in_=ident_d[:, :])
        wp_sb = [per.tile([HD, DIM], MM_DT, tag=f"wp{h}", name=f"wp{h}")
                 for h in range(HPC)]

        # persistent transposed q,k [hd, S] and attention output [hd, S];
        # v natural per 512-superchunk with the ones column for denominators
        qkT = [per.tile([HD, S], MM_DT, tag=f"qkT{j}", name=f"qkT{j}")
               for j in range(2 * HPC)]
        att_o = [per.tile([HD, S], MM_DT, tag=f"atto{h}", name=f"atto{h}")
                 for h in range(HPC)]
        v_sb = [[per.tile([128, 4, VEXT], MM_DT, tag=f"v{h}_{ss}",
                          name=f"v{h}_{ss}") for ss in range(NSS)]
                for h in range(HPC)]

        # one shared PSUM pool (8 bank-sized slots shared by every phase so
        # the scheduler can overlap A/B/C), plus top-level SBUF pools
        psp = ctx.enter_context(tc.tile_pool(name="ps", bufs=5, space="PSUM"))
        qkvp = ctx.enter_context(tc.tile_pool(name="qkvsb", bufs=3))
        ropep = ctx.enter_context(tc.tile_pool(name="ropet", bufs=2))
        qkrop = ctx.enter_context(tc.tile_pool(name="qkro", bufs=3))
        expp = ctx.enter_context(tc.tile_pool(name="expp", bufs=5))
        smp = ctx.enter_context(tc.tile_pool(name="smalls", bufs=2))
        outp = ctx.enter_context(tc.tile_pool(name="outp", bufs=2))

        if nbm:
            bmask_sb = per.tile([128, nbm], MM_DT, tag="bmask")
            nc.sync.dma_start(out=bmask_sb,
                              in_=bmask_d.ap().rearrange("n p -> p n"))
            bidx = {(j, r0, r1): i for i, (j, r0, r1) in enumerate(bpairs)}

        # ---------------- phase bodies (emitted interleaved below) --------
        def emit_A(ss):
            """QKV + RoPE + transposes for s-superchunk ss."""
            xt = xts[ss]
            if ss > 0:
                nc.sync.dma_start(
                    out=xt,
                    in_=xblk_d[ss].rearrange("sub p dp c n -> p sub dp c n"))
            if ss == 1:
                for h in range(HPC):
                    nc.sync.dma_start(out=wp_sb[h],
                                      in_=wpT_d[h * HD:(h + 1) * HD, :])
            for h in range(HPC):
                # zero pad + ones column for the softmax-denominator trick
                nc.gpsimd.memset(v_sb[h][ss][:, :, HD:VEXT - 1], 0.0)
                nc.gpsimd.memset(v_sb[h][ss][:, :, VEXT - 1:VEXT], 1.0)
            # bf16 transposes packed two tensor-heads per PSUM bank
            tp2 = [psp.tile([HD, 2, 512], MM_DT, tag="tp", bufs=2, name="tpps")
                   for _ in range(HPC)]
            nh = 2 * HPC  # 4 roped qk tensor-heads
            for sub in range(4):
                qp = psp.tile([128, ODIM], F32, tag="qp", bufs=1, name="qkvps")
                for d in range(10):
                    nc.tensor.matmul(
                        qp[:, :],
                        lhsT=xt[:, sub, d // 2, d % 2, :],
                        rhs=wblk_sb[:, d, :], start=(d == 0), stop=(d == 9))

                # single PSUM->SBUF evacuation fused with the bias add
                # (bias pre-replicated across partitions host-side); RoPE
                # and the v copies then run from bf16 SBUF
                qsb = qkvp.tile([128, ODIM], MM_DT, tag="qkv", name="qsb")
                with nc.allow_low_precision("bf16 evac"):
                    nc.scalar.copy(qsb, qp)
                    nc.vector.tensor_add(qsb, qsb, brow_sb)

                # RoPE over q,k: out = t*cos + halfswap(t)*sinsgn
                m1 = ropep.tile([128, QKDIM], MM_DT, tag="m1")
                m2 = ropep.tile([128, QKDIM], MM_DT, tag="m2")
                qk_h = qsb[:, 0:QKDIM].rearrange("p (h d) -> p h d", h=nh)
                cos_b = cos_sb[:, ss, sub:sub + 1, :].to_broadcast(
                    [128, nh, HD])
                nc.vector.tensor_mul(
                    m1.rearrange("p (h d) -> p h d", h=nh), qk_h, cos_b)
                swap = qsb[:, 0:QKDIM].rearrange(
                    "p (h x d) -> p h x d", h=nh, x=2)[:, :, ::-1, :]
                sin_b = sin_sb[:, ss, sub:sub + 1, :].rearrange(
                    "p t (x d) -> p (t x) d", x=2)[:, None, :, :] \
                    .to_broadcast([128, nh, 2, HD // 2])
                nc.vector.tensor_mul(
                    m2.rearrange("p (h x d) -> p h x d", h=nh, x=2),
                    swap, sin_b)
                ro = qkrop.tile([128, QKDIM], MM_DT, tag="qkro")
                nc.vector.tensor_add(ro, m1, m2)

                # v natural copies are SBUF->SBUF now: Pool engine
                for h in range(HPC):
                    nc.gpsimd.tensor_copy(
                        v_sb[h][ss][:, sub, 0:HD],
                        qsb[:, QKDIM + HD * h:QKDIM + HD * (h + 1)])

                # transpose roped q,k to [hd, s]
                for j in range(2 * HPC):
                    nc.tensor.transpose(
                        tp2[j // 2][:, j % 2, 128 * sub:128 * (sub + 1)],
                        ro[:, HD * j:HD * (j + 1)], ident_sb)
            for j in range(2 * HPC):
                with nc.allow_low_precision("bf16 matmul inputs"):
                    nc.vector.tensor_copy(
                        qkT[j][:, 512 * ss:512 * (ss + 1)],
                        tp2[j // 2][:, j % 2, :])

        # stage-2 (normalize) closures deferred one group so the PE has the
        # next group's score/attn matmuls to chew on while the DVE computes
        # the reciprocal of this group's denominator row.
        pending_B: list = []

        def flush_B(keep: int = 0):
            while len(pending_B) > keep:
                pending_B.pop(0)()

        def emit_B(seg, on_progress=None):
            """block-diagonal attention for one segment (both heads)."""
            s0, s1 = seg
            jb0, jb1 = s0 // 128, -(-s1 // 128)
            # q-chunks of <=512 (one fp32 PSUM bank per score matmul)
            qchunks = []
            g = s0
            while g < s1:
                q1 = min(s1, (g - g % 2) + 512)  # even-widened span <= 512
                qchunks.append((g, q1))
                g = q1
            for q0, q1 in qchunks:
                # widen to even alignment for bf16 matmul moving dims
                qa0 = q0 - (q0 % 2)
                qa1 = min(S, q1 + (q1 % 2))
                qna, off, qn = qa1 - qa0, q0 - qa0, q1 - q0
                for h in range(HPC):
                    ap_ = psp.tile([VEXT, 512], F32, tag="ps", name="attps")
                    blocks = list(range(jb0, jb1))
                    # pairs of k-blocks; a narrow (<=256) pair shares one
                    # PSUM bank and gets a single merged exp
                    for g0 in range(0, len(blocks), 2):
                        grp = blocks[g0:g0 + 2]
                        merged = len(grp) == 2 and qna <= 256
                        if merged:
                            sc = psp.tile([128, 2, 256], F32, tag="ps",
                                          name="scps")
                            scs = [sc[:, i, :qna] for i in range(2)]
                        else:
                            scs = [psp.tile([128, 512], F32, tag="ps",
                                            name="scps")[:, :qna]
                                   for _ in grp]
                        for i, j in enumerate(grp):
                            nc.tensor.matmul(
                                scs[i],
                                lhsT=qkT[HPC + h][:, 128 * j:128 * (j + 1)],
                                rhs=qkT[h][:, qa0:qa1],
                                start=True, stop=True)
                        if merged:
                            ex = expp.tile([128, 2, 256], MM_DT, tag="expp")
                            exs = [ex[:, i, :qna] for i in range(2)]
                            nc.scalar.activation(
                                ex[:, :, :qna], sc[:, :, :qna],
                                mybir.ActivationFunctionType.Exp)
                        else:
                            exs = []
                            for i in range(len(grp)):
                                ex = expp.tile([128, 512], MM_DT, tag="expp")
                                nc.scalar.activation(
                                    ex[:, :qna], scs[i],
                                    mybir.ActivationFunctionType.Exp)
                                exs.append(ex[:, :qna])
                        for i, j in enumerate(grp):
                            r0, r1 = max(0, s0 - 128 * j), min(128, s1 - 128 * j)
                            if r0 > 0 or r1 < 128:
                                # zero out-of-segment rows of this block
                                mi = bidx[(j, r0, r1)]
                                with nc.allow_low_precision("bf16 inputs"):
                                    nc.vector.tensor_mul(
                                        exs[i], exs[i],
                                        bmask_sb[:, mi:mi + 1]
                                        .to_broadcast([128, qna]))
                        for i, j in enumerate(grp):
                            nc.tensor.matmul(
                                ap_[:, :qna],
                                lhsT=v_sb[h][j // 4][:, j % 4, :],
                                rhs=exs[i],
                                start=(j == blocks[0]),
                                stop=(j == blocks[-1]))

                    def stage2(ap_=ap_, h=h, q0=q0, q1=q1, qna=qna, off=off,
                               qn=qn):
                        den = smp.tile([1, 512], MM_DT, tag="den", name="den")
                        with nc.allow_low_precision("bf16 matmul inputs"):
                            nc.vector.tensor_copy(den[:, :qna],
                                                  ap_[96:97, :qna])
                        bc = psp.tile([HD, 512], F32, tag="ps", name="bcps")
                        nc.tensor.matmul(bc[:, :qna], lhsT=ones_sb[:, 0:HD],
                                         rhs=den[:, :qna],
                                         start=True, stop=True)
                        rec = smp.tile([HD, 512], MM_DT, tag="rec", name="rec")
                        with nc.allow_low_precision("bf16 matmul inputs"):
                            nc.vector.reciprocal(rec[:, :qna], bc[:, :qna])
                            nc.vector.tensor_mul(att_o[h][:, q0:q1],
                                                 ap_[0:HD, off:off + qn],
                                                 rec[:, off:off + qn])

                    pending_B.append(stage2)
                    flush_B(keep=1)
                if on_progress:
                    on_progress(q1)

        def emit_C(sc_):
            """projection for output s-superchunk sc_.
            b_proj is added host-side after the cross-core partial sum."""
            for mh in range(2):
                ob = outp.tile([128, 5, 512], MM_DT, tag="outp")
                for mm_ in range(5):
                    m = 5 * mh + mm_
                    pp = psp.tile([128, 512], F32, tag="ps", name="prps")
                    for h in range(HPC):
                        nc.tensor.matmul(
                            pp[:, :],
                            lhsT=wp_sb[h][:, 128 * m:128 * (m + 1)],
                            rhs=att_o[h][:, 512 * sc_:512 * (sc_ + 1)],
                            start=(h == 0), stop=(h == HPC - 1))
                    with nc.allow_low_precision("bf16 output"):
                        if mm_ % 2 == 0:
                            nc.vector.tensor_copy(ob[:, mm_, :], pp)
                        else:
                            nc.scalar.copy(ob[:, mm_, :], pp)
                if sc_ == NSS - 1:
                    # final superchunk: small SP-issued pieces so the tail
                    # transfer after the last copy is minimal
                    for p0, p1 in ((0, 2), (2, 4), (4, 5)):
                        nc.sync.dma_start(out=outb_d[sc_, mh, :, p0:p1],
                                          in_=ob[:, p0:p1, :])
                else:
                    nc.gpsimd.dma_start(out=outb_d[sc_, mh, :, 0:3],
                                        in_=ob[:, 0:3, :])
                    nc.gpsimd.dma_start(out=outb_d[sc_, mh, :, 3:5],
                                        in_=ob[:, 3:5, :])

        # ---- interleaved driver: emit B as soon as its span is produced,
        # ---- C as soon as all segments covering its chunk are attended.
        segs_left = sorted(segments, key=lambda s: s[1])
        c_next = 0

        def process(done_to, frontier_cb=None):
            nonlocal c_next
            while segs_left and segs_left[0][1] <= done_to:
                emit_B(segs_left.pop(0), on_progress=frontier_cb)
            covered = min((s0 for (s0, s1) in segs_left), default=S)
            if c_next < NSS and 512 * (c_next + 1) <= covered:
                flush_B()
                while c_next < NSS and 512 * (c_next + 1) <= covered:
                    emit_C(c_next)
                    c_next += 1

        for ss in range(NSS):
            emit_A(ss)
            if ss >= 1:
                process(512 * ss)
        process(S)
        assert not segs_left
        flush_B()
        while c_next < NSS:
            emit_C(c_next)
            c_next += 1

    nc.compile()
    return nc


def _prep_inputs(x, cu_seqlens, rotary_pos_emb, w_qkv, b_qkv, w_proj, b_proj):
    """Host-side shard prep. Returns per-core input dicts."""
    import ml_dtypes
    bf16 = ml_dtypes.bfloat16
    scale = np.float32(1.0 / np.sqrt(np.float32(HD)))
    xT = np.asarray(x, np.float32).T
    w_qkv = np.asarray(w_qkv, np.float32)
    b_qkv = np.asarray(b_qkv, np.float32)
    w_proj = np.asarray(w_proj, np.float32)
    rot = np.asarray(rotary_pos_emb, np.float32)

    cosw = np.concatenate([np.cos(rot), np.cos(rot)], axis=1).astype(np.float32)
    sinw = np.concatenate([-np.sin(rot), np.sin(rot)], axis=1).astype(np.float32)
    # blocked layouts so every device DMA reads one dense contiguous region:
    # xblk[ss,sub,p,dp,c,n] = xT[256dp+128c+p, 512ss+128sub+n]
    xblk = np.ascontiguousarray(
        xT.reshape(5, 2, 128, 6, 4, 128).transpose(3, 4, 2, 0, 1, 5)) \
        .astype(bf16)
    # cosb[p,ss,t,d] = cosw[512ss+128t+p, d]
    cosb = np.ascontiguousarray(
        cosw.reshape(6, 4, 128, HD).transpose(2, 0, 1, 3)).astype(bf16)
    sinb = np.ascontiguousarray(
        sinw.reshape(6, 4, 128, HD).transpose(2, 0, 1, 3)).astype(bf16)
    ident = np.eye(128, dtype=np.float32).astype(bf16)
    onesrow = np.ones((1, 128), dtype=np.float32).astype(bf16)
    vpad = np.zeros(17, dtype=np.float32)
    vpad[16] = 1.0
    vpad = vpad.astype(bf16)
    segments = _segments_from_cu(cu_seqlens)
    bmask_rows = []
    for (j, r0, r1) in _bpairs(segments):
        row = np.zeros(128, dtype=np.float32)
        row[r0:r1] = 1.0
        bmask_rows.append(row)
    bmask = (np.stack(bmask_rows).astype(bf16) if bmask_rows else None)

    in_maps = []
    for c in range(NCORES):
        heads = [HPC * c + i for i in range(HPC)]
        idx = []
        for base in (0, DIM, 2 * DIM):           # q, k, v row blocks
            for h in heads:
                idx.extend(range(base + h * HD, base + (h + 1) * HD))
        w_c = w_qkv[idx, :].copy()
        b_c = b_qkv[idx].copy()
        w_c[:QKDIM // 2] *= scale                # scale q by 1/sqrt(HD)
        b_c[:QKDIM // 2] *= scale
        # wblk[p, d, o] = w_c.T[128d+p, o]
        wblk = np.ascontiguousarray(
            w_c.T.reshape(10, 128, ODIM).transpose(1, 0, 2)).astype(bf16)
        cdims = []
        for h in heads:
            cdims.extend(range(h * HD, (h + 1) * HD))
        wpT = np.ascontiguousarray(w_proj[:, cdims].T).astype(bf16)
        in_maps.append({
            "xblk": xblk,
            "wblk": wblk,
            "bqkv": np.ascontiguousarray(
                np.broadcast_to(b_c[None, :], (128, ODIM))).astype(bf16),
            "cosb": cosb,
            "sinb": sinb,
            "wpT": wpT,
            "ident": ident,
            "onesrow": onesrow,
            "vpad": vpad,
        })
        if bmask is not None:
            in_maps[-1]["bmask"] = bmask
    return in_maps


def run(inputs: dict, trace: bool = False):
    """Build (cached), run on 8 cores, return (out [S, DIM] fp32, results)."""
    segments = _segments_from_cu(inputs["cu_seqlens"])
    key = (segments, str(MM_DT))
    if key not in _CACHE:
        _CACHE[key] = _build(segments)
    nc = _CACHE[key]
    in_maps = _prep_inputs(
        inputs["x"], inputs["cu_seqlens"], inputs["rotary_pos_emb"],
        inputs["w_qkv"], inputs["b_qkv"], inputs["w_proj"], inputs["b_proj"])
    res = run_bass_kernel_spmd(nc, in_maps, core_ids=list(range(NCORES)),
                               trace=trace)
    acc = np.zeros((DIM, S), np.float64)
    for r in res.results:
        # blocked [sc, mh, p, c, n] -> [dim = 640*mh+128*c+p, s = 512*sc+n]
        acc += r["outb"].astype(np.float64).transpose(1, 3, 2, 0, 4) \
            .reshape(DIM, S)
    acc += np.asarray(inputs["b_proj"], np.float64)[:, None]
    out = np.ascontiguousarray(acc.T.astype(np.float32))
    return out, res


def kernel(**inputs) -> np.ndarray:
    out, _ = run(inputs, trace=False)
    return out
